# revision 1
# baseline (speedup 1.0000x reference)
"""GCNConv-with-constraint kernel for 8 Trainium2 NeuronCores.

Strategy (per the sharding hint): nodes are sharded across the 8 cores by
destination; edges are partitioned by destination shard so the scatter-add is
core-local. Each core holds a replicated fp16 copy of x (the gather table) and
the small 128x128 weight.

Per-core device pipeline:
  prologue: column-renorm of W on device (matmul for column norms ->
            sqrt/recip/min -> broadcast -> transpose) giving WnT [in, out].
  edge phase (bulk, memory-bound): batched SWDGE dma_gather of x[src] rows
            (fp16, thousands of rows per call; int16 indices so edges are
            pre-split into lo/hi table halves) -> DVE builds norm-scaled
            one-hots sel[e,t,d] = (iota==dst_e,t)*norm_e,t for a whole call
            in two wide tensor_tensor ops -> PE matmul aggT[ch,dst] +=
            msgs^T @ sel accumulated in PSUM fp32 per 128-destination block.
  block epilogue: aggT -> SBUF, fp32 matmul with WnT (the linear layer),
            + bias, DMA out as out^T [128ch, dst].

Host does only structure/metadata work: degree bincount -> dinv, per-edge
norm values (the CSR values of the normalized adjacency), self-loop append,
sort/pad edges by destination block and table half, fp16 cast of x, and the
final transpose/concat of the 8 output shards.
"""

import math
import os
from contextlib import ExitStack

import numpy as np

import concourse.bass as bass
import concourse.tile as tile
from concourse import bacc, mybir
from concourse.bass_utils import run_bass_kernel_spmd

N_CORES = 8
C = 128  # in/out channels
P = 128  # partitions / block size
HALF = 32768  # int16-addressable rows per gather table half
TILES_PER_CALL = 64  # max tiles per dma_gather call
GROUP = 6  # destination blocks per superblock (PSUM live set)

f16 = mybir.dt.float16
f32 = mybir.dt.float32
i16 = mybir.dt.int16

# test.py introspection: the last BassKernelResults (exec_time_ns when traced)
LAST_RESULTS = None


def _prep(x, edge_index, W, b):
    """Host-side sharding/metadata prep. Returns per-core input maps and
    the (data-dependent) structure baked into the program."""
    N = x.shape[0]
    npc = math.ceil(N / N_CORES)  # nodes per core (last shard may be partial)
    n_blocks = math.ceil(npc / P)

    src = np.asarray(edge_index[0], dtype=np.int64)
    dst = np.asarray(edge_index[1], dtype=np.int64)

    deg = np.bincount(dst, minlength=N).astype(np.float64) + 1.0
    dinv = 1.0 / np.sqrt(deg)
    norm = dinv[src] * dinv[dst]

    # self loops
    ar = np.arange(N, dtype=np.int64)
    src_all = np.concatenate([src, ar])
    dst_all = np.concatenate([dst, ar])
    norm_all = np.concatenate([norm, dinv * dinv]).astype(np.float32)

    shard = dst_all // npc
    dst_loc = dst_all - shard * npc
    blk = dst_loc >> 7
    ishi = (src_all >= HALF).astype(np.int64)
    key = blk * 2 + ishi  # (block, table-half) bucket
    nk = n_blocks * 2

    # per-(shard, key) counts decide the common padded tile structure
    cnt_sk = np.zeros((N_CORES, nk), dtype=np.int64)
    for s in range(N_CORES):
        m = shard == s
        cnt_sk[s] = np.bincount(key[m], minlength=nk)
    tiles_k = (cnt_sk.max(axis=0) + P - 1) // P  # tiles per (block, half)

    # stream order: superblocks of GROUP blocks; within one, all lo tiles
    # then all hi tiles (each dma_gather call must be half-homogeneous)
    stream_keys = []  # key order in the tile stream
    runs = []  # key list per (superblock, half) run
    for g0 in range(0, n_blocks, GROUP):
        g1 = min(g0 + GROUP, n_blocks)
        for half in (0, 1):
            r = [2 * bb + half for bb in range(g0, g1) if tiles_k[2 * bb + half] > 0]
            if r:
                runs.append(r)
                stream_keys.extend(r)
    stream_pos = np.zeros(nk, dtype=np.int64)  # key -> first tile index
    t = 0
    for kkey in stream_keys:
        stream_pos[kkey] = t
        t += tiles_k[kkey]
    n_tiles = int(t)

    # per-tile program structure
    block_of = np.zeros(n_tiles, dtype=np.int64)
    half_of = np.zeros(n_tiles, dtype=np.int64)
    for kkey in stream_keys:
        t0, tn = stream_pos[kkey], tiles_k[kkey]
        block_of[t0 : t0 + tn] = kkey // 2
        half_of[t0 : t0 + tn] = kkey % 2
    first_of = np.zeros(n_tiles, dtype=bool)
    last_of = np.zeros(n_tiles, dtype=bool)
    for bb in range(n_blocks):
        ts = np.where(block_of == bb)[0]
        assert len(ts) > 0
        first_of[ts.min()] = True
        last_of[ts.max()] = True

    # gather calls: equal chunks of each run, capped at TILES_PER_CALL
    calls = []  # (t0, kg, half)
    for r in runs:
        t0 = int(stream_pos[r[0]])
        L = int(sum(tiles_k[kkey] for kkey in r))
        nch = math.ceil(L / TILES_PER_CALL)
        sizes = [L // nch + (1 if i < L % nch else 0) for i in range(nch)]
        o = t0
        for sz in sizes:
            calls.append((o, sz, int(r[0] % 2)))
            o += sz

    xtab = np.ascontiguousarray(x.astype(np.float16))
    iota_arr = np.ascontiguousarray(
        np.broadcast_to(
            np.tile(np.arange(P, dtype=np.float16), TILES_PER_CALL), (P, TILES_PER_CALL * P)
        )
    )
    ident = np.eye(C, dtype=np.float32)
    wf = np.ascontiguousarray(np.asarray(W, dtype=np.float32))
    bvec = np.ascontiguousarray(np.asarray(b, dtype=np.float32).reshape(C, 1))

    pad_off = stream_pos * P  # key -> padded edge offset

    in_maps = []
    for s in range(N_CORES):
        m = shard == s
        sl_src = src_all[m] - HALF * ishi[m]
        sl_dl = dst_loc[m]
        sl_key = key[m]
        sl_nm = norm_all[m]
        order = np.argsort(sl_key, kind="stable")
        cnt = np.bincount(sl_key, minlength=nk)
        starts = np.concatenate([[0], np.cumsum(cnt)[:-1]])
        pos_in_grp = np.arange(len(sl_src)) - np.repeat(starts, cnt)
        dst_pos = np.repeat(pad_off, cnt) + pos_in_grp

        S = np.zeros(n_tiles * P, np.int16)
        D = np.full(n_tiles * P, -1.0, np.float16)
        NM = np.zeros(n_tiles * P, np.float16)
        S[dst_pos] = sl_src[order].astype(np.int16)
        D[dst_pos] = (sl_dl[order] & 127).astype(np.float16)
        NM[dst_pos] = sl_nm[order].astype(np.float16)

        # dma_gather wrapped-16 index layout: idx i at [i%16, i//16],
        # replicated across the eight 16-partition groups
        srcw16 = S.reshape(n_tiles * 8, 16).T  # [16, 8*n_tiles]
        srcw = np.ascontiguousarray(np.tile(srcw16, (8, 1)))  # [128, 8*n_tiles]

        in_maps.append(
            {
                "xtab": xtab,
                "srcw": srcw,
                "dsts": np.ascontiguousarray(D.reshape(n_tiles, P).T),
                "nrms": np.ascontiguousarray(NM.reshape(n_tiles, P).T),
                "w": wf,
                "bvec": bvec,
                "iota": iota_arr,
                "ident": ident,
            }
        )

    structure = dict(
        N=N,
        npc=npc,
        n_blocks=n_blocks,
        n_tiles=n_tiles,
        calls=calls,
        block_of=block_of,
        half_of=half_of,
        first_of=first_of,
        last_of=last_of,
    )
    return in_maps, structure


def _build_program(st, repeat=1):
    N, n_tiles, n_blocks = st["N"], st["n_tiles"], st["n_blocks"]
    nc = bacc.Bacc("TRN2", target_bir_lowering=False, debug=False, num_devices=N_CORES)

    xtab = nc.dram_tensor("xtab", [N, C], f16, kind="ExternalInput").ap()
    srcw = nc.dram_tensor("srcw", [P, 8 * n_tiles], i16, kind="ExternalInput").ap()
    dsts = nc.dram_tensor("dsts", [P, n_tiles], f16, kind="ExternalInput").ap()
    nrms = nc.dram_tensor("nrms", [P, n_tiles], f16, kind="ExternalInput").ap()
    w = nc.dram_tensor("w", [C, C], f32, kind="ExternalInput").ap()
    bvec = nc.dram_tensor("bvec", [C, 1], f32, kind="ExternalInput").ap()
    iota = nc.dram_tensor("iota", [P, TILES_PER_CALL * P], f16, kind="ExternalInput").ap()
    ident = nc.dram_tensor("ident", [C, C], f32, kind="ExternalInput").ap()
    outt = nc.dram_tensor("outt", [C, n_blocks * P], f32, kind="ExternalOutput").ap()

    Copy = mybir.ActivationFunctionType.Copy
    Sqrt = mybir.ActivationFunctionType.Sqrt
    Op = mybir.AluOpType

    with tile.TileContext(nc) as tc, ExitStack() as ctx:
        cpool = ctx.enter_context(tc.tile_pool(name="const", bufs=1))
        iota_sb = cpool.tile([P, TILES_PER_CALL, P], f16, tag="iota")
        wnT_sb = cpool.tile([C, C], f32, tag="wnT")
        bias_sb = cpool.tile([C, 1], f32, tag="bias")
        nc.sync.dma_start(iota_sb[:], iota[:])  # [P, KMAX, P] tiled iota
        nc.sync.dma_start(bias_sb[:], bvec[:])

        # ---- prologue: Wn = W * min(1, 1/||W[:,i]||); WnT = Wn^T ----
        with (
            tc.tile_pool(name="prol", bufs=1) as pp,
            tc.tile_pool(name="prol_ps", bufs=1, space="PSUM") as ppp,
        ):
            w_sb = pp.tile([C, C], f32, tag="w")
            nc.sync.dma_start(w_sb[:], w[:])
            ident_sb = pp.tile([C, C], f32, tag="ident")
            nc.sync.dma_start(ident_sb[:], ident[:])
            wsq = pp.tile([C, C], f32, tag="wsq")
            nc.vector.tensor_tensor(out=wsq[:], in0=w_sb[:], in1=w_sb[:], op=Op.mult)
            ones_c = pp.tile([C, 1], f32, tag="ones_c")
            nc.vector.memset(ones_c[:], 1.0)
            cn_ps = ppp.tile([1, C], f32, tag="cn")
            nc.tensor.matmul(cn_ps[:], lhsT=ones_c[:], rhs=wsq[:], start=True, stop=True)
            nrm_sb = pp.tile([1, C], f32, tag="nrm")
            nc.scalar.activation(nrm_sb[:], cn_ps[:], Sqrt)
            rec_sb = pp.tile([1, C], f32, tag="rec")
            nc.vector.reciprocal(rec_sb[:], nrm_sb[:])
            scl_sb = pp.tile([1, C], f32, tag="scl")
            nc.vector.tensor_scalar(
                out=scl_sb[:], in0=rec_sb[:], scalar1=1.0, scalar2=None, op0=Op.min
            )
            ones_r = pp.tile([1, C], f32, tag="ones_r")
            nc.vector.memset(ones_r[:], 1.0)
            sbc_ps = ppp.tile([C, C], f32, tag="sbc")
            nc.tensor.matmul(
                sbc_ps[:], lhsT=ones_r[:], rhs=scl_sb[:], start=True, stop=True
            )
            wn_sb = pp.tile([C, C], f32, tag="wn")
            nc.vector.tensor_tensor(out=wn_sb[:], in0=w_sb[:], in1=sbc_ps[:], op=Op.mult)
            wnT_ps = ppp.tile([C, C], f32, tag="wnT_ps")
            nc.tensor.matmul(
                wnT_ps[:], lhsT=wn_sb[:], rhs=ident_sb[:], start=True, stop=True
            )
            nc.scalar.activation(wnT_sb[:], wnT_ps[:], Copy)

        # ---- edge phase ----
        mpool = ctx.enter_context(tc.tile_pool(name="meta", bufs=3))
        gpool = ctx.enter_context(tc.tile_pool(name="gather", bufs=3))
        spool = ctx.enter_context(tc.tile_pool(name="sel", bufs=2))
        apool = ctx.enter_context(tc.tile_pool(name="aggsb", bufs=2))
        opool = ctx.enter_context(tc.tile_pool(name="outsb", bufs=2))
        agg_psp = ctx.enter_context(tc.tile_pool(name="aggps", bufs=6, space="PSUM"))
        out_psp = ctx.enter_context(tc.tile_pool(name="outps", bufs=1, space="PSUM"))

        xtab_hi = xtab[HALF:, :] if N > HALF else None
        block_of, first_of, last_of = st["block_of"], st["first_of"], st["last_of"]
        for _rep in range(repeat):
          agg_ps_of_block = {}
          for t0, kg, half in st["calls"]:
              src_sl = mpool.tile([P, kg * 8], i16, tag="srcsl")
              nc.sync.dma_start(src_sl[:], srcw[:, 8 * t0 : 8 * (t0 + kg)])
              dst_sl = mpool.tile([P, kg], f16, tag="dstsl")
              nc.sync.dma_start(dst_sl[:], dsts[:, t0 : t0 + kg])
              nrm_sl = mpool.tile([P, kg], f16, tag="nrmsl")
              nc.sync.dma_start(nrm_sl[:], nrms[:, t0 : t0 + kg])
              gbuf = gpool.tile([P, kg, C], f16, tag="gbuf")
              nc.gpsimd.dma_gather(
                  out_ap=gbuf[:],
                  in_ap=(xtab[:] if half == 0 else xtab_hi),
                  idxs_ap=src_sl[:],
                  num_idxs=kg * P,
                  num_idxs_reg=kg * P,
                  elem_size=C,
                  single_packet=False,
              )
              # packed norm-scaled one-hot for all kg tiles in two wide DVE ops
              sel3 = spool.tile([P, TILES_PER_CALL, P], f16, tag="sel")
              nc.vector.tensor_tensor(
                  out=sel3[:, :kg, :],
                  in0=iota_sb[:, :kg, :],
                  in1=dst_sl[:].unsqueeze(2).to_broadcast([P, kg, P]),
                  op=Op.is_equal,
              )
              nc.vector.tensor_tensor(
                  out=sel3[:, :kg, :],
                  in0=sel3[:, :kg, :],
                  in1=nrm_sl[:].unsqueeze(2).to_broadcast([P, kg, P]),
                  op=Op.mult,
              )
              for slot in range(kg):
                  t = t0 + slot
                  b = int(block_of[t])
                  if first_of[t]:
                      agg_ps_of_block[b] = agg_psp.tile([C, P], f32, tag="aggps", name=f"aggps_b{b}")
                  agg_ps = agg_ps_of_block[b]
                  nc.tensor.matmul(
                      agg_ps[:],
                      lhsT=gbuf[:, slot, :],
                      rhs=sel3[:, slot, :],
                      start=bool(first_of[t]),
                      stop=bool(last_of[t]),
                  )
                  if last_of[t]:
                      del agg_ps_of_block[b]
                      agg_sb = apool.tile([C, P], f32, tag="aggsb")
                      nc.scalar.activation(agg_sb[:], agg_ps[:], Copy)
                      outT_ps = out_psp.tile([C, P], f32, tag="outps")
                      nc.tensor.matmul(
                          outT_ps[:], lhsT=wnT_sb[:], rhs=agg_sb[:], start=True, stop=True
                      )
                      outT_sb = opool.tile([C, P], f32, tag="outsb")
                      nc.vector.tensor_scalar(
                          out=outT_sb[:],
                          in0=outT_ps[:],
                          scalar1=bias_sb[:],
                          scalar2=None,
                          op0=Op.add,
                      )
                      nc.sync.dma_start(outt[:, b * P : (b + 1) * P], outT_sb[:])

    nc.compile()
    return nc


def kernel(x, edge_index, W, b):
    global LAST_RESULTS
    x = np.asarray(x)
    N = x.shape[0]
    assert x.shape[1] == C and W.shape == (C, C)

    in_maps, st = _prep(x, edge_index, W, b)
    nc = _build_program(st)

    os.environ.setdefault("BASS_NEVER_TRACE", "1")  # no NTFF hook in this env
    res = run_bass_kernel_spmd(nc, in_maps, list(range(N_CORES)))
    LAST_RESULTS = res

    npc = st["npc"]
    shards = []
    for s in range(N_CORES):
        lo = s * npc
        hi = min((s + 1) * npc, N)
        outt = res.results[s]["outt"]  # [C, n_blocks*P]
        shards.append(outt[:, : hi - lo].T)
    return np.ascontiguousarray(np.concatenate(shards, axis=0), dtype=np.float32)



# revision 5
# speedup vs baseline: 14.2187x; 14.2187x over previous
"""GCNConv-with-constraint kernel for 8 Trainium2 NeuronCores.

This environment charges a large fixed cost (~40-120us) per dynamically
executed engine instruction, so the kernel is designed to minimize dynamic
instruction count; FLOPs and bytes are comparatively free.

Math: out = D^-1/2 (A+I) D^-1/2 (x @ Wn^T) + b, with Wn = column-renormed W.
Since the linear layer commutes with aggregation and the symmetric norm is
separable (norm_e = dinv[src]*dinv[dst]), we aggregate xs = dinv[src]*x rows
(host-prescaled fp16), apply dinv[dst] on the aggregate, then one 128x128
linear on the aggregate.

Sharding: destination nodes across the 8 cores (6250 dsts/core); each core
holds the full xs gather table (lo/hi int16-addressable halves, each with a
trailing zero row used for padding slots).

Device pipeline per core (one program, SPMD):
  prologue: column-renorm of W on device -> WnT [in,out] fp32 (as baseline).
  edge phase: destinations are degree-sorted and packed into a handful of
    chunks; every dst in a chunk gets a fixed slot count (D_lo lo-half slots,
    D_hi hi-half slots; padding slots index the zero row). Per chunk:
    2 idx DMAs + 2 TRANSPOSED dma_gathers producing msgsT [ch, dst, slot]
    fp16 + 2 DVE tensor_reduce (slot axis, fp32 out) + 1 add -> aggT[ch,dst].
  epilogue: aggT *= dinvrep (1 DVE op), 13 fp32 matmuls WnT@aggT (512-wide),
    bias-add copies, one output DMA of outT [128, Wtot] fp32.

Host does structure/metadata only: degree bincount -> dinv, xs prescale/cast,
degree sorting, chunk packing, slot-table fill, and the final
transpose/unpermute of the 8 output shards.
"""

import math
import os
from contextlib import ExitStack

import numpy as np

import concourse.bass as bass
import concourse.tile as tile
from concourse import bacc, mybir
from concourse.bass_utils import run_bass_kernel_spmd

N_CORES = 8
C = 128  # in/out channels
P = 128  # partitions
SPLIT = 32767  # lo-table rows (zero row at index SPLIT); hi zero row at end
SLOT_BUDGET = 49152  # (lo+hi) slots per chunk: 96KB/lane of fp16 gather buf
WCOL = 512  # output column tile (matmul moving width, fp32)

f16 = mybir.dt.float16
f32 = mybir.dt.float32
i16 = mybir.dt.int16

# test.py introspection: the last BassKernelResults
LAST_RESULTS = None
GMAX = 14336  # max idxs per transposed dma_gather (descriptor carveout limit)


def _wrap16(S):
    """int16 idx stream (len % 128 == 0) -> wrapped-16 [128, len/16] layout."""
    nt = len(S) // 128
    w16 = S.reshape(nt * 8, 16).T  # [16, 8*nt]
    return np.ascontiguousarray(np.tile(w16, (8, 1)))


def _even_up(v):
    return int(v + (v & 1))


def _prep(x, edge_index, W, b):
    N = x.shape[0]
    npc = math.ceil(N / N_CORES)
    split = min(N, SPLIT)

    src = np.asarray(edge_index[0], dtype=np.int64)
    dst = np.asarray(edge_index[1], dtype=np.int64)

    deg = np.bincount(dst, minlength=N).astype(np.float64) + 1.0
    dinv = 1.0 / np.sqrt(deg)
    xs = (np.asarray(x, dtype=np.float64) * dinv[:, None]).astype(np.float16)

    xlo = np.vstack([xs[:split], np.zeros((1, C), np.float16)])
    xhi = np.vstack([xs[split:], np.zeros((1, C), np.float16)])
    nhi = xhi.shape[0]  # zero row at nhi-1

    ar = np.arange(N, dtype=np.int64)
    src_all = np.concatenate([src, ar])
    dst_all = np.concatenate([dst, ar])
    shard = dst_all // npc
    dloc = dst_all - shard * npc
    islo = src_all < split

    # per-(core, dst) lo/hi counts
    flat = shard * npc + dloc
    lo_cnt = np.bincount(flat[islo], minlength=N_CORES * npc).reshape(N_CORES, npc)
    hi_cnt = np.bincount(flat[~islo], minlength=N_CORES * npc).reshape(N_CORES, npc)

    # degree-sort dsts per core (desc by total count), rank envelope across cores
    tot = lo_cnt + hi_cnt
    order = np.argsort(-tot, axis=1, kind="stable")  # [cores, npc] rank -> dloc
    rank = np.empty_like(order)
    np.put_along_axis(rank, order, np.arange(npc)[None, :].repeat(N_CORES, 0), axis=1)
    env_lo = np.take_along_axis(lo_cnt, order, axis=1).max(axis=0)  # [npc]
    env_hi = np.take_along_axis(hi_cnt, order, axis=1).max(axis=0)

    # chunk packing over ranks: n (mult of 64) dsts per chunk, common D_lo/D_hi
    chunks = []  # (n, dl, dh) ; dummies pad the tail
    i = 0
    while i < npc:
        dl = _even_up(max(1, env_lo[i : i + 64].max()))
        dh = _even_up(env_hi[i : i + 64].max())
        while True:
            n = max(64, (SLOT_BUDGET // max(1, dl + dh)) & ~63)
            dl2 = _even_up(max(1, env_lo[i : i + n].max()))
            dh2 = _even_up(env_hi[i : i + n].max())
            if dl2 == dl and dh2 == dh:
                break
            dl, dh = dl2, dh2
        n = min(n, ((npc - i) + 63) & ~63)
        chunks.append([n, dl, dh])
        i += n
    # pad total columns to a multiple of WCOL (extra dummy dsts in last chunk)
    wtot = sum(c[0] for c in chunks)
    extra = (-wtot) % WCOL
    chunks[-1][0] += extra
    wtot += extra

    col_off = np.cumsum([0] + [c[0] for c in chunks])[:-1]
    lo_base = np.cumsum([0] + [c[0] * c[1] for c in chunks])  # idx stream offsets
    hi_base = np.cumsum([0] + [c[0] * c[2] for c in chunks])
    Llo, Lhi = int(lo_base[-1]), int(hi_base[-1])

    # rank -> (chunk id, column, slot-base)
    cid_of_rank = np.zeros(npc, np.int64)
    col_of_rank = np.zeros(npc, np.int64)
    lo_slot0 = np.zeros(npc, np.int64)
    hi_slot0 = np.zeros(npc, np.int64)
    r0 = 0
    for ci, (n, dl, dh) in enumerate(chunks):
        r1 = min(r0 + n, npc)
        rr = np.arange(r0, r1)
        cid_of_rank[rr] = ci
        col_of_rank[rr] = col_off[ci] + (rr - r0)
        lo_slot0[rr] = lo_base[ci] + (rr - r0) * dl
        hi_slot0[rr] = hi_base[ci] + (rr - r0) * dh
        r0 += n

    wf = np.ascontiguousarray(np.asarray(W, dtype=np.float32))
    bvec = np.ascontiguousarray(np.asarray(b, dtype=np.float32).reshape(C, 1))
    ident = np.eye(C, dtype=np.float32)

    in_maps = []
    perms = []  # per core: global dst id per column (-1 dummy)
    for s in range(N_CORES):
        m = shard == s
        e_src = src_all[m]
        e_r = rank[s][dloc[m]]
        e_lo = islo[m]

        LO = np.full(Llo, split, np.int16)
        HI = np.full(Lhi, nhi - 1, np.int16)
        for half, tab in ((True, LO), (False, HI)):
            sel = e_lo == half
            rr = e_r[sel]
            ss = e_src[sel] if half else e_src[sel] - split
            o = np.argsort(rr, kind="stable")
            rr, ss = rr[o], ss[o]
            # position within each rank group
            cnts = np.bincount(rr, minlength=npc)
            starts = np.concatenate([[0], np.cumsum(cnts)[:-1]])
            pos = np.arange(len(rr)) - starts[rr]
            base = lo_slot0 if half else hi_slot0
            tab[base[rr] + pos] = ss.astype(np.int16)

        ncols_real = npc
        perm = np.full(wtot, -1, np.int64)
        lim = min(s * npc + npc, N) - s * npc  # real dsts this core
        valid = np.arange(ncols_real)[:lim] if lim < npc else np.arange(npc)
        perm[col_of_rank[valid]] = s * npc + order[s][valid]
        perms.append(perm)

        dvals = np.zeros(wtot, np.float32)
        ok = perm >= 0
        dvals[ok] = dinv[perm[ok]].astype(np.float32)
        dinvrep = np.ascontiguousarray(np.broadcast_to(dvals[None, :], (P, wtot)))

        im = {
            "xlo": xlo,
            "xhi": xhi,
            "dinvrep": dinvrep,
            "w": wf,
            "bvec": bvec,
            "ident": ident,
        }
        if Llo:
            im["srcwlo"] = _wrap16(LO)
        if Lhi:
            im["srcwhi"] = _wrap16(HI)
        in_maps.append(im)

    st = dict(
        N=N,
        npc=npc,
        split=split,
        nhi=nhi,
        chunks=chunks,
        wtot=wtot,
        Llo=Llo,
        Lhi=Lhi,
        lo_base=lo_base,
        hi_base=hi_base,
        col_off=col_off,
        perms=perms,
    )
    return in_maps, st


def _build_program(st, repeat=1):
    N, wtot, chunks = st["N"], st["wtot"], st["chunks"]
    Llo, Lhi = st["Llo"], st["Lhi"]
    nc = bacc.Bacc("TRN2", target_bir_lowering=False, debug=False, num_devices=N_CORES)

    xlo = nc.dram_tensor("xlo", [st["split"] + 1, C], f16, kind="ExternalInput").ap()
    xhi = nc.dram_tensor("xhi", [st["nhi"], C], f16, kind="ExternalInput").ap()
    srcwlo = (
        nc.dram_tensor("srcwlo", [P, Llo // 16], i16, kind="ExternalInput").ap()
        if Llo
        else None
    )
    srcwhi = (
        nc.dram_tensor("srcwhi", [P, Lhi // 16], i16, kind="ExternalInput").ap()
        if Lhi
        else None
    )
    dinvrep = nc.dram_tensor("dinvrep", [P, wtot], f32, kind="ExternalInput").ap()
    w = nc.dram_tensor("w", [C, C], f32, kind="ExternalInput").ap()
    bvec = nc.dram_tensor("bvec", [C, 1], f32, kind="ExternalInput").ap()
    ident = nc.dram_tensor("ident", [C, C], f32, kind="ExternalInput").ap()
    outt = nc.dram_tensor("outt", [C, wtot], f32, kind="ExternalOutput").ap()

    Copy = mybir.ActivationFunctionType.Copy
    Sqrt = mybir.ActivationFunctionType.Sqrt
    Op = mybir.AluOpType
    AX = mybir.AxisListType.X

    with tile.TileContext(nc) as tc, ExitStack() as ctx:
        cpool = ctx.enter_context(tc.tile_pool(name="const", bufs=1))
        wnT_sb = cpool.tile([C, C], f32, tag="wnT")
        bias_sb = cpool.tile([C, 1], f32, tag="bias")
        dinv_sb = cpool.tile([P, wtot], f32, tag="dinv")
        nc.sync.dma_start(bias_sb[:], bvec[:])
        nc.sync.dma_start(dinv_sb[:], dinvrep[:])

        # ---- prologue: Wn = W * min(1, 1/||W[:,i]||); WnT = Wn^T ----
        with (
            tc.tile_pool(name="prol", bufs=1) as pp,
            tc.tile_pool(name="prol_ps", bufs=1, space="PSUM") as ppp,
        ):
            w_sb = pp.tile([C, C], f32, tag="w")
            nc.sync.dma_start(w_sb[:], w[:])
            ident_sb = pp.tile([C, C], f32, tag="ident")
            nc.sync.dma_start(ident_sb[:], ident[:])
            wsq = pp.tile([C, C], f32, tag="wsq")
            nc.vector.tensor_tensor(out=wsq[:], in0=w_sb[:], in1=w_sb[:], op=Op.mult)
            ones_c = pp.tile([C, 1], f32, tag="ones_c")
            nc.vector.memset(ones_c[:], 1.0)
            cn_ps = ppp.tile([1, C], f32, tag="cn")
            nc.tensor.matmul(cn_ps[:], lhsT=ones_c[:], rhs=wsq[:], start=True, stop=True)
            nrm_sb = pp.tile([1, C], f32, tag="nrm")
            nc.scalar.activation(nrm_sb[:], cn_ps[:], Sqrt)
            rec_sb = pp.tile([1, C], f32, tag="rec")
            nc.vector.reciprocal(rec_sb[:], nrm_sb[:])
            scl_sb = pp.tile([1, C], f32, tag="scl")
            nc.vector.tensor_scalar(
                out=scl_sb[:], in0=rec_sb[:], scalar1=1.0, scalar2=None, op0=Op.min
            )
            ones_r = pp.tile([1, C], f32, tag="ones_r")
            nc.vector.memset(ones_r[:], 1.0)
            sbc_ps = ppp.tile([C, C], f32, tag="sbc")
            nc.tensor.matmul(
                sbc_ps[:], lhsT=ones_r[:], rhs=scl_sb[:], start=True, stop=True
            )
            wn_sb = pp.tile([C, C], f32, tag="wn")
            nc.vector.tensor_tensor(out=wn_sb[:], in0=w_sb[:], in1=sbc_ps[:], op=Op.mult)
            wnT_ps = ppp.tile([C, C], f32, tag="wnT_ps")
            nc.tensor.matmul(
                wnT_ps[:], lhsT=wn_sb[:], rhs=ident_sb[:], start=True, stop=True
            )
            nc.scalar.activation(wnT_sb[:], wnT_ps[:], Copy)

        # ---- edge + output phase ----
        mpool = ctx.enter_context(tc.tile_pool(name="meta", bufs=2))
        glopool = ctx.enter_context(tc.tile_pool(name="glo", bufs=1))
        ghipool = ctx.enter_context(tc.tile_pool(name="ghi", bufs=1))
        tpool = ctx.enter_context(tc.tile_pool(name="tmp", bufs=2))
        apool = ctx.enter_context(tc.tile_pool(name="aggT", bufs=1))
        opool = ctx.enter_context(tc.tile_pool(name="outsb", bufs=1))
        out_psp = ctx.enter_context(tc.tile_pool(name="outps", bufs=4, space="PSUM"))

        for _rep in range(repeat):
            aggT = apool.tile([P, wtot], f32, tag="aggT")
            for ci, (n, dl, dh) in enumerate(chunks):
                co = int(st["col_off"][ci])
                agg_slice = aggT[:, co : co + n]
                halves = []
                if dl > 0:
                    halves.append((srcwlo, st["lo_base"][ci], n * dl, dl, xlo, "lo"))
                if dh > 0:
                    halves.append((srcwhi, st["hi_base"][ci], n * dh, dh, xhi, "hi"))
                red_tgts = []
                for srcw, base, L, dd, xtab, hname in halves:
                    gp = glopool if hname == "lo" else ghipool
                    gbuf = gp.tile([P, n, dd], f16, tag=f"g{hname}")
                    gflat = gbuf[:].rearrange("p a b -> p (a b)")
                    ncalls = -(-L // GMAX)
                    per = -(-(L // 128) // ncalls) * 128  # multiple of 128
                    for a in range(0, L, per):
                        b = min(a + per, L)
                        idx_sl = mpool.tile([P, (b - a) // 16], i16, tag=f"idx{hname}")
                        nc.sync.dma_start(
                            idx_sl[:],
                            srcw[:, (int(base) + a) // 16 : (int(base) + b) // 16],
                        )
                        nc.gpsimd.dma_gather(
                            out_ap=gflat[:, a:b].unsqueeze(1),
                            in_ap=xtab[:],
                            idxs_ap=idx_sl[:],
                            num_idxs=b - a,
                            num_idxs_reg=b - a,
                            elem_size=C,
                            transpose=True,
                            single_packet=False,
                        )
                    red_tgts.append(gbuf)
                if len(red_tgts) == 2:
                    nc.vector.tensor_reduce(
                        out=agg_slice, in_=red_tgts[0][:], axis=AX, op=Op.add
                    )
                    tmp = tpool.tile([P, n], f32, tag="tmp")
                    nc.vector.tensor_reduce(
                        out=tmp[:], in_=red_tgts[1][:], axis=AX, op=Op.add
                    )
                    nc.vector.tensor_tensor(
                        out=agg_slice, in0=agg_slice, in1=tmp[:], op=Op.add
                    )
                else:
                    nc.vector.tensor_reduce(
                        out=agg_slice, in_=red_tgts[0][:], axis=AX, op=Op.add
                    )

            # dst-side norm factor
            nc.vector.tensor_tensor(
                out=aggT[:], in0=aggT[:], in1=dinv_sb[:], op=Op.mult
            )

            # linear layer + bias, 512 columns at a time
            out_sb = opool.tile([P, wtot], f32, tag="outsb")
            for j in range(wtot // WCOL):
                ps = out_psp.tile([C, WCOL], f32, tag="ops")
                nc.tensor.matmul(
                    ps[:],
                    lhsT=wnT_sb[:],
                    rhs=aggT[:, j * WCOL : (j + 1) * WCOL],
                    start=True,
                    stop=True,
                )
                nc.vector.tensor_scalar(
                    out=out_sb[:, j * WCOL : (j + 1) * WCOL],
                    in0=ps[:],
                    scalar1=bias_sb[:],
                    scalar2=None,
                    op0=Op.add,
                )
            nc.sync.dma_start(outt[:], out_sb[:])

    nc.compile()
    return nc


def kernel(x, edge_index, W, b):
    global LAST_RESULTS
    x = np.asarray(x)
    N = x.shape[0]
    assert x.shape[1] == C and W.shape == (C, C)

    in_maps, st = _prep(x, edge_index, W, b)
    nc = _build_program(st)

    os.environ.setdefault("BASS_NEVER_TRACE", "1")  # no NTFF hook in this env
    res = run_bass_kernel_spmd(nc, in_maps, list(range(N_CORES)))
    LAST_RESULTS = res

    out = np.zeros((N, C), np.float32)
    for s in range(N_CORES):
        outt = res.results[s]["outt"]  # [C, wtot]
        perm = st["perms"][s]
        ok = perm >= 0
        out[perm[ok]] = outt[:, ok].T
    return np.ascontiguousarray(out)
